# revision 24
# baseline (speedup 1.0000x reference)
"""Trainium2 Bass kernel for nn_NonsharedPatchEmbed_86827058856432.

Computes, for a patchified [64, 3, 224, 224] fp32 image batch,

    out[b, p, o] = sum_i patches[b, p, i] * W[p, o, i] + bias[p, o]

with 196 independent Linear(768->768) layers (one per patch).

Distribution: the 196-patch axis is sharded across the 8 NeuronCores, 25
patches per core (tail padded with patch 0, dropped on the host). Patch-
parallel reads W exactly once, which is the traffic roofline; data-parallel
over batch would read all of W on every core.

The kernel is SBUF-fabric/DMA-bound on W traffic, so W rides as few bytes as
accuracy allows: contraction chunks 0-3 (512 of 768 inputs) in bf16, chunks
4-5 in fp8e4m3 (1.67 B/elem average, 24.6 MB/core). Host-measured rel err of
this split is 1.52e-2 vs the 2e-2 gate (HW matched to 5 digits). To keep fp8
in a good exponent range while mixing exactly with the bf16 chunks,
everything accumulates out/128 in PSUM: bf16 chunks store W/128, fp8 chunks
store e4m3(W*128) and use a second stationary a*2^-14 (computed on-device by
DVE), bias is pre-divided by 128, and the PSUM->SBUF copy multiplies by 128.
All scale factors are powers of two, exact in bf16/fp32.

Per-core kernel (column-tiled pairs):
  - 13 pairs of patches; the 13th processes the single last patch on PSUM
    rows 0-63 only. Patch A owns PSUM partitions 0-63 (tile_position (0,0)),
    patch B owns 64-127 ((0,64)); each streams its own W as the moving
    operand, the shared batch activations (aT chunks [128 x 64]) are
    stationary. Matmuls alternate positions (A,B,A,B) so consecutive streams
    overlap on the PE's column tiles.
  - The fp32 bias is applied exactly via a K=2 bf16 matmul (ones x [hi;lo])
    that starts each PSUM accumulation group, absorbing the PSUM WAR
    dependency.
  - W is laid out pair-major so each pair needs just TWO DMAs (bf16 portion
    1.57 MB at 12.3 KB/partition, fp8 portion 393 KB at 3 KB/partition),
    alternating rings per pair to balance SP/ACT; activations and biases
    ride one upfront DMA each; outputs (bf16) ride ACT.

Layouts per core:
  aT   [128, 25, 6, 64]     bf16  aT[i, p, c, b] = patches[b, 25k+p, 128c+i]
  Wbf  [13, 128, 2, 4, 768] bf16  Wbf[j, i, u, c, o] = W[25k+2j+u, o, 128c+i]/128
  Wf8  [13, 128, 2, 2, 768] f8e4  Wf8[j, i, u, c, o] = e4m3(W[25k+2j+u, o, 512+128c+i]*128)
  bhl  [2, 25, 768]         bf16  bias/128 split as hi + lo
  outp [13, 128, 768]       bf16  pair j rows 0-63 -> patch 2j, 64-127 -> 2j+1
  (pair 12 duplicates patch 24 in the Wbf/Wf8 layout; only rows 0-63 used)
"""

import numpy as np
import ml_dtypes

import concourse.tile as tile
import concourse.mybir as mybir
from concourse import bacc
from concourse.bass_utils import run_bass_kernel_spmd

f32 = mybir.dt.float32
bf16 = mybir.dt.bfloat16
f8e4 = mybir.dt.float8e4

N_CORES = 8
B = 64            # batch
D = 768           # in/out feature dim
NP = 196          # real patches
PPC = 25          # patches per core (8*25 = 200, tail padded)
NCHUNK = 6        # 768 / 128 contraction chunks
NBF = 4           # chunks 0..3 in bf16
NF8 = 2           # chunks 4..5 in fp8e4m3
NPAIR = PPC // 2 + 1   # 12 real pairs + 1 single-last-patch "pair"

LAST_RESULTS = None    # BassKernelResults of the most recent run (for test.py)

_NC_CACHE = {}


def _build():
    nc = bacc.Bacc()
    aT = nc.declare_dram_parameter("aT", [128, PPC, NCHUNK, B], bf16, isOutput=False)
    Wbf = nc.declare_dram_parameter(
        "Wbf", [NPAIR, 128, 2, NBF, D], bf16, isOutput=False)
    Wf8 = nc.declare_dram_parameter(
        "Wf8", [NPAIR, 128, 2, NF8, D], f8e4, isOutput=False)
    bhl = nc.declare_dram_parameter("bhl", [2, PPC, D], bf16, isOutput=False)
    outp = nc.declare_dram_parameter("outp", [NPAIR, 2 * B, D], bf16, isOutput=True)

    with tile.TileContext(nc) as tc:
        with (
            tc.tile_pool(name="const", bufs=1) as cpool,
            tc.tile_pool(name="w", bufs=7) as wpool,
            tc.tile_pool(name="w8", bufs=7) as w8pool,
            tc.tile_pool(name="o", bufs=4) as opool,
            tc.tile_pool(name="ps", bufs=4, space="PSUM") as pspool,
        ):
            ones = cpool.tile([2, B], bf16)
            nc.vector.memset(ones[:], 1.0)

            at = cpool.tile([128, PPC, NCHUNK, B], bf16)
            nc.sync.dma_start(at[:], aT[:])
            bt = cpool.tile([2, PPC, D], bf16)
            nc.scalar.dma_start(bt[:], bhl[:])
            # second stationary for the fp8 chunks: a * 2^-14 (exact scale)
            a2 = cpool.tile([128, PPC, NF8, B], bf16)

            slices = [(0, 512), (512, 768)]

            for j in range(NPAIR):
                lastpair = j == NPAIR - 1
                nu = 1 if lastpair else 2
                wt = wpool.tile([128, nu, NBF, D], bf16, tag="wt")
                w8 = w8pool.tile([128, nu, NF8, D], f8e4, tag="w8")
                e0, e1 = (nc.sync, nc.scalar) if j % 2 == 0 else (nc.scalar, nc.sync)
                e0.dma_start(wt[:], Wbf[j, :, :nu])
                e1.dma_start(w8[:], Wf8[j, :, :nu])

                if not lastpair:
                    p0, p1 = 2 * j, 2 * j + 1
                    # (patch, w-slot, psum rows, output column range)
                    positions = [(p0, 0, 0, B, 0, D), (p1, 1, B, 2 * B, 0, D)]
                    for p in (p0, p1):
                        nc.vector.tensor_scalar_mul(
                            a2[:, p], at[:, p, NBF:], 2.0 ** -14
                        )
                else:
                    # single last patch: split its output columns across the
                    # two PE column-tile positions to halve the serial tail
                    p0 = p1 = PPC - 1
                    positions = [(p0, 0, 0, B, 0, 512), (p0, 0, B, 2 * B, 512, D)]
                    nc.vector.tensor_scalar_mul(
                        a2[:, p0], at[:, p0, NBF:], 2.0 ** -14
                    )

                pt = pspool.tile([2 * B, D], f32, tag="pt")
                for (o0, o1) in slices:
                    for (p, u, r0, r1, q0, q1) in positions:
                        if o0 >= q1 or o1 <= q0:
                            continue
                        nc.tensor.matmul(
                            pt[r0:r1, o0:o1], ones[:], bt[:, p, o0:o1],
                            start=True, stop=False, tile_position=(0, r0),
                        )
                for c in range(NCHUNK):
                    last = c == NCHUNK - 1
                    for (o0, o1) in slices:
                        for (p, u, r0, r1, q0, q1) in positions:
                            if o0 >= q1 or o1 <= q0:
                                continue
                            if c < NBF:
                                stat, mov = at[:, p, c, :], wt[:, u, c, o0:o1]
                            else:
                                stat, mov = a2[:, p, c - NBF, :], w8[:, u, c - NBF, o0:o1]
                            nc.tensor.matmul(
                                pt[r0:r1, o0:o1], stat, mov,
                                start=False, stop=last, tile_position=(0, r0),
                            )
                ob = opool.tile([2 * B, D], bf16, tag="ob")
                if not lastpair:
                    nc.vector.tensor_scalar_mul(ob[:], pt[:], 128.0)
                    nc.scalar.dma_start(outp[j], ob[:])
                else:
                    # rows 0-63 hold cols 0-512, rows 64-127 hold cols 512-768.
                    # Copy the two live quadrants, then ship ONE full-width
                    # 128-partition DMA: narrow (64-partition) transfers land
                    # on few DMA engines and pay a multi-us completion tail.
                    nc.vector.tensor_scalar_mul(
                        ob[:B, :512], pt[:B, :512], 128.0)
                    nc.vector.tensor_scalar_mul(
                        ob[B:, 512:], pt[B:, 512:], 128.0)
                    nc.scalar.dma_start(outp[j], ob[:])

    nc.finalize()
    return nc


def _patchify(x):
    # [B, C, H, W] -> [B, 196, 768] in MAE ordering (n c h p w q -> n h w p q c)
    Bn, C, H, Wd = x.shape
    h = H // 16
    xr = x.reshape(Bn, C, h, 16, h, 16)
    xr = np.transpose(xr, (0, 2, 4, 3, 5, 1))
    return xr.reshape(Bn, h * h, 16 * 16 * C)


def kernel(x, W, b, _trace=False):
    global LAST_RESULTS

    x = np.asarray(x, dtype=np.float32)
    W = np.asarray(W, dtype=np.float32)
    b = np.asarray(b, dtype=np.float32)

    patches = _patchify(x)                      # [64, 196, 768]

    # pair-major patch index per core: [13, 2] with the last pair = [24, 24]
    pidx = np.empty((NPAIR, 2), dtype=np.int64)
    pidx[:NPAIR - 1, 0] = np.arange(0, PPC - 1, 2)
    pidx[:NPAIR - 1, 1] = np.arange(1, PPC, 2)
    pidx[NPAIR - 1] = PPC - 1

    in_maps = []
    for k in range(N_CORES):
        idx = np.arange(k * PPC, (k + 1) * PPC)
        idx[idx >= NP] = 0                      # pad tail with patch 0
        psl = patches[:, idx, :]                # [64, 25, 768]
        wsl = W[idx]                            # [25, 768, 768]
        bsl = b[idx] * (1.0 / 128.0)            # bias pre-divided (exact)

        aT = np.ascontiguousarray(
            psl.transpose(2, 1, 0)              # [768(i), 25, 64]
            .reshape(NCHUNK, 128, PPC, B)
            .transpose(1, 2, 0, 3)              # [128, 25, 6, 64]
        ).astype(ml_dtypes.bfloat16)
        Wt = (
            wsl.transpose(0, 2, 1)              # [25, 768(i), 768(o)]
            .reshape(PPC, NCHUNK, 128, D)
            .transpose(0, 2, 1, 3)              # [25, 128, 6, 768]
        )
        Wp = Wt[pidx]                            # [13, 2, 128, 6, 768]
        Wp = Wp.transpose(0, 2, 1, 3, 4)         # [13, 128, 2, 6, 768]
        Wb = np.ascontiguousarray(Wp[:, :, :, :NBF] * (1.0 / 128.0)).astype(
            ml_dtypes.bfloat16
        )
        W8 = np.ascontiguousarray(Wp[:, :, :, NBF:] * 128.0).astype(
            ml_dtypes.float8_e4m3fn
        )
        hi = bsl.astype(ml_dtypes.bfloat16)
        lo = (bsl - hi.astype(np.float32)).astype(ml_dtypes.bfloat16)
        bhl = np.ascontiguousarray(np.stack([hi, lo], axis=0))
        in_maps.append({"aT": aT, "Wbf": Wb, "Wf8": W8, "bhl": bhl})

    if "F" not in _NC_CACHE:
        _NC_CACHE["F"] = _build()
    nc = _NC_CACHE["F"]

    res = run_bass_kernel_spmd(nc, in_maps, list(range(N_CORES)), trace=_trace)
    LAST_RESULTS = res

    # outp [13, 128, 768] per core: pair rows -> patches; last pair -> rows 0:64
    parts = []
    for k in range(N_CORES):
        op = res.results[k]["outp"].astype(np.float32)
        full = np.empty((PPC, B, D), dtype=np.float32)
        full[:PPC - 1] = op[:NPAIR - 1].reshape(PPC - 1, B, D)
        # last patch: cols 0-512 from rows 0-63, cols 512-768 from rows 64-127
        full[PPC - 1, :, :512] = op[NPAIR - 1, :B, :512]
        full[PPC - 1, :, 512:] = op[NPAIR - 1, B:, 512:]
        parts.append(full[None])
    parts = np.concatenate(parts)               # [8, 25, 64, 768]
    full = parts.transpose(2, 0, 1, 3).reshape(B, N_CORES * PPC, D)
    return np.ascontiguousarray(full[:, :NP, :])


# revision 25
# speedup vs baseline: 1.1468x; 1.1468x over previous
"""Trainium2 Bass kernel for nn_NonsharedPatchEmbed_86827058856432.

Computes, for a patchified [64, 3, 224, 224] fp32 image batch,

    out[b, p, o] = sum_i patches[b, p, i] * W[p, o, i] + bias[p, o]

with 196 independent Linear(768->768) layers (one per patch).

Distribution: the 196-patch axis is sharded across the 8 NeuronCores, 25
patches per core (tail padded with patch 0, dropped on the host). Patch-
parallel reads W exactly once, which is the traffic roofline; data-parallel
over batch would read all of W on every core.

The kernel is SBUF-fabric/DMA-bound on W traffic, so W rides as few bytes as
accuracy allows: contraction chunks 0-3 (512 of 768 inputs) in bf16, chunks
4-5 in fp8e4m3 (1.67 B/elem average, 24.6 MB/core). Host-measured rel err of
this split is 1.52e-2 vs the 2e-2 gate (HW matched to 5 digits). To keep fp8
in a good exponent range while mixing exactly with the bf16 chunks,
everything accumulates out/128 in PSUM: bf16 chunks store W/128, fp8 chunks
store e4m3(W*128) and use a second stationary a*2^-14 (computed on-device by
DVE), bias is pre-divided by 128, and the PSUM->SBUF copy multiplies by 128.
All scale factors are powers of two, exact in bf16/fp32.

Per-core kernel (column-tiled pairs):
  - 13 pairs of patches; the 13th processes the single last patch on PSUM
    rows 0-63 only. Patch A owns PSUM partitions 0-63 (tile_position (0,0)),
    patch B owns 64-127 ((0,64)); each streams its own W as the moving
    operand, the shared batch activations (aT chunks [128 x 64]) are
    stationary. Matmuls alternate positions (A,B,A,B) so consecutive streams
    overlap on the PE's column tiles.
  - The fp32 bias is applied exactly via a K=2 bf16 matmul (ones x [hi;lo])
    that starts each PSUM accumulation group, absorbing the PSUM WAR
    dependency.
  - W is laid out pair-major so each pair needs just TWO DMAs (bf16 portion
    1.57 MB at 12.3 KB/partition, fp8 portion 393 KB at 3 KB/partition),
    alternating rings per pair to balance SP/ACT; activations and biases
    ride one upfront DMA each; outputs (bf16) ride ACT.

Layouts per core:
  aT   [128, 25, 6, 64]     bf16  aT[i, p, c, b] = patches[b, 25k+p, 128c+i]
  Wbf  [13, 128, 2, 4, 768] bf16  Wbf[j, i, u, c, o] = W[25k+2j+u, o, 128c+i]/128
  Wf8  [13, 128, 2, 2, 768] f8e4  Wf8[j, i, u, c, o] = e4m3(W[25k+2j+u, o, 512+128c+i]*128)
  bhl  [2, 25, 768]         bf16  bias/128 split as hi + lo
  outp [13, 128, 768]       bf16  pair j rows 0-63 -> patch 2j, 64-127 -> 2j+1
  (pair 12 duplicates patch 24 in the Wbf/Wf8 layout; only rows 0-63 used)
"""

import numpy as np
import ml_dtypes

import concourse.tile as tile
import concourse.mybir as mybir
from concourse import bacc
from concourse.bass_utils import run_bass_kernel_spmd

f32 = mybir.dt.float32
bf16 = mybir.dt.bfloat16
f8e4 = mybir.dt.float8e4

N_CORES = 8
B = 64            # batch
D = 768           # in/out feature dim
NP = 196          # real patches
PPC = 25          # patches per core (8*25 = 200, tail padded)
NCHUNK = 6        # 768 / 128 contraction chunks
NBF = 4           # chunks 0..3 in bf16
NF8 = 2           # chunks 4..5 in fp8e4m3
NPAIR = PPC // 2 + 1   # 12 real pairs + 1 single-last-patch "pair"

LAST_RESULTS = None    # BassKernelResults of the most recent run (for test.py)

_NC_CACHE = {}


def _build():
    nc = bacc.Bacc()
    aT = nc.declare_dram_parameter("aT", [128, PPC, NCHUNK, B], bf16, isOutput=False)
    Wbf = nc.declare_dram_parameter(
        "Wbf", [NPAIR, 128, 2, NBF, D], bf16, isOutput=False)
    Wf8 = nc.declare_dram_parameter(
        "Wf8", [NPAIR, 128, 2, NF8, D], f8e4, isOutput=False)
    bhl = nc.declare_dram_parameter("bhl", [2, PPC, D], bf16, isOutput=False)
    outp = nc.declare_dram_parameter("outp", [NPAIR, 2 * B, D], bf16, isOutput=True)

    with tile.TileContext(nc) as tc:
        with (
            tc.tile_pool(name="const", bufs=1) as cpool,
            tc.tile_pool(name="w", bufs=7) as wpool,
            tc.tile_pool(name="w8", bufs=7) as w8pool,
            tc.tile_pool(name="o", bufs=4) as opool,
            tc.tile_pool(name="ps", bufs=4, space="PSUM") as pspool,
        ):
            ones = cpool.tile([2, B], bf16)
            nc.vector.memset(ones[:], 1.0)

            at = cpool.tile([128, PPC, NCHUNK, B], bf16)
            nc.sync.dma_start(at[:], aT[:])
            bt = cpool.tile([2, PPC, D], bf16)
            nc.scalar.dma_start(bt[:], bhl[:])
            # second stationary for the fp8 chunks: a * 2^-14 (exact scale)
            a2 = cpool.tile([128, PPC, NF8, B], bf16)

            slices = [(0, 512), (512, 768)]

            for j in range(NPAIR):
                lastpair = j == NPAIR - 1
                nu = 1 if lastpair else 2
                wt = wpool.tile([128, nu, NBF, D], bf16, tag="wt")
                w8 = w8pool.tile([128, nu, NF8, D], f8e4, tag="w8")
                e0, e1 = (nc.sync, nc.scalar) if j % 2 == 0 else (nc.scalar, nc.sync)
                e0.dma_start(wt[:], Wbf[j, :, :nu])
                e1.dma_start(w8[:], Wf8[j, :, :nu])

                if not lastpair:
                    p0, p1 = 2 * j, 2 * j + 1
                    # (patch, w-slot, psum rows, output column range)
                    positions = [(p0, 0, 0, B, 0, D), (p1, 1, B, 2 * B, 0, D)]
                    for p in (p0, p1):
                        nc.vector.tensor_scalar_mul(
                            a2[:, p], at[:, p, NBF:], 2.0 ** -14
                        )
                else:
                    # single last patch: split its output columns across the
                    # two PE column-tile positions to halve the serial tail
                    p0 = p1 = PPC - 1
                    positions = [(p0, 0, 0, B, 0, 512), (p0, 0, B, 2 * B, 512, D)]
                    nc.vector.tensor_scalar_mul(
                        a2[:, p0], at[:, p0, NBF:], 2.0 ** -14
                    )

                pt = pspool.tile([2 * B, D], f32, tag="pt")
                for (o0, o1) in slices:
                    for (p, u, r0, r1, q0, q1) in positions:
                        if o0 >= q1 or o1 <= q0:
                            continue
                        nc.tensor.matmul(
                            pt[r0:r1, o0:o1], ones[:], bt[:, p, o0:o1],
                            start=True, stop=False, tile_position=(0, r0),
                        )
                for c in range(NCHUNK):
                    last = c == NCHUNK - 1
                    for (o0, o1) in slices:
                        for (p, u, r0, r1, q0, q1) in positions:
                            if o0 >= q1 or o1 <= q0:
                                continue
                            if c < NBF:
                                stat, mov = at[:, p, c, :], wt[:, u, c, o0:o1]
                            else:
                                stat, mov = a2[:, p, c - NBF, :], w8[:, u, c - NBF, o0:o1]
                            nc.tensor.matmul(
                                pt[r0:r1, o0:o1], stat, mov,
                                start=False, stop=last, tile_position=(0, r0),
                            )
                ob = opool.tile([2 * B, D], bf16, tag="ob")
                if not lastpair:
                    nc.vector.tensor_scalar_mul(ob[:], pt[:], 128.0)
                    nc.scalar.dma_start(outp[j], ob[:])
                else:
                    # rows 0-63 hold cols 0-512, rows 64-127 hold cols 512-768
                    nc.vector.tensor_scalar_mul(
                        ob[:B, :512], pt[:B, :512], 128.0)
                    nc.scalar.dma_start(outp[j, :B, :512], ob[:B, :512])
                    nc.vector.tensor_scalar_mul(
                        ob[B:, 512:], pt[B:, 512:], 128.0)
                    nc.scalar.dma_start(outp[j, B:, 512:], ob[B:, 512:])

    nc.finalize()
    return nc


def _patchify(x):
    # [B, C, H, W] -> [B, 196, 768] in MAE ordering (n c h p w q -> n h w p q c)
    Bn, C, H, Wd = x.shape
    h = H // 16
    xr = x.reshape(Bn, C, h, 16, h, 16)
    xr = np.transpose(xr, (0, 2, 4, 3, 5, 1))
    return xr.reshape(Bn, h * h, 16 * 16 * C)


def kernel(x, W, b, _trace=False):
    global LAST_RESULTS

    x = np.asarray(x, dtype=np.float32)
    W = np.asarray(W, dtype=np.float32)
    b = np.asarray(b, dtype=np.float32)

    patches = _patchify(x)                      # [64, 196, 768]

    # pair-major patch index per core: [13, 2] with the last pair = [24, 24]
    pidx = np.empty((NPAIR, 2), dtype=np.int64)
    pidx[:NPAIR - 1, 0] = np.arange(0, PPC - 1, 2)
    pidx[:NPAIR - 1, 1] = np.arange(1, PPC, 2)
    pidx[NPAIR - 1] = PPC - 1

    in_maps = []
    for k in range(N_CORES):
        idx = np.arange(k * PPC, (k + 1) * PPC)
        idx[idx >= NP] = 0                      # pad tail with patch 0
        psl = patches[:, idx, :]                # [64, 25, 768]
        wsl = W[idx]                            # [25, 768, 768]
        bsl = b[idx] * (1.0 / 128.0)            # bias pre-divided (exact)

        aT = np.ascontiguousarray(
            psl.transpose(2, 1, 0)              # [768(i), 25, 64]
            .reshape(NCHUNK, 128, PPC, B)
            .transpose(1, 2, 0, 3)              # [128, 25, 6, 64]
        ).astype(ml_dtypes.bfloat16)
        Wt = (
            wsl.transpose(0, 2, 1)              # [25, 768(i), 768(o)]
            .reshape(PPC, NCHUNK, 128, D)
            .transpose(0, 2, 1, 3)              # [25, 128, 6, 768]
        )
        Wp = Wt[pidx]                            # [13, 2, 128, 6, 768]
        Wp = Wp.transpose(0, 2, 1, 3, 4)         # [13, 128, 2, 6, 768]
        Wb = np.ascontiguousarray(Wp[:, :, :, :NBF] * (1.0 / 128.0)).astype(
            ml_dtypes.bfloat16
        )
        W8 = np.ascontiguousarray(Wp[:, :, :, NBF:] * 128.0).astype(
            ml_dtypes.float8_e4m3fn
        )
        hi = bsl.astype(ml_dtypes.bfloat16)
        lo = (bsl - hi.astype(np.float32)).astype(ml_dtypes.bfloat16)
        bhl = np.ascontiguousarray(np.stack([hi, lo], axis=0))
        in_maps.append({"aT": aT, "Wbf": Wb, "Wf8": W8, "bhl": bhl})

    if "F" not in _NC_CACHE:
        _NC_CACHE["F"] = _build()
    nc = _NC_CACHE["F"]

    res = run_bass_kernel_spmd(nc, in_maps, list(range(N_CORES)), trace=_trace)
    LAST_RESULTS = res

    # outp [13, 128, 768] per core: pair rows -> patches; last pair -> rows 0:64
    parts = []
    for k in range(N_CORES):
        op = res.results[k]["outp"].astype(np.float32)
        full = np.empty((PPC, B, D), dtype=np.float32)
        full[:PPC - 1] = op[:NPAIR - 1].reshape(PPC - 1, B, D)
        # last patch: cols 0-512 from rows 0-63, cols 512-768 from rows 64-127
        full[PPC - 1, :, :512] = op[NPAIR - 1, :B, :512]
        full[PPC - 1, :, 512:] = op[NPAIR - 1, B:, 512:]
        parts.append(full[None])
    parts = np.concatenate(parts)               # [8, 25, 64, 768]
    full = parts.transpose(2, 0, 1, 3).reshape(B, N_CORES * PPC, D)
    return np.ascontiguousarray(full[:, :NP, :])
